# revision 24
# baseline (speedup 1.0000x reference)
"""CoAttention module kernel for Trainium2 (8 NeuronCores).

Problem: B=4 pairs of (left, right) feature maps [B, C=2048, H=W=48].
Two attention directions per pair -> 8 independent attention problems,
one per core (data parallel, no cross-core communication).

Per core (qf = query features [C, HW], rf = reference features [C, HW]):
    Q = Wq @ qf + bq          [HC=256, HW=2304]   (fp32r matmuls)
    K = Wk @ rf + bk          [HC=256, HW=2304]
    S = Q^T K                 [2304, 2304]        (fp32r)
    P = softmax(S, axis=-1)                       (exact row max, ACT exp)
    O = V P^T, V = rf         [C, HW]             (bf16 matmuls)

Schedule (emission order == per-engine execution order):
  Phase R (DMA-bound): rf streamed once in 512-col stripes; K projection
    (fp32r, PSUM-accumulated over the 16 channel chunks) plus the bf16
    V-transpose (PE transpose, hidden under the DMA stream). W^T tiles
    are built first from DMAs issued on the ACT HWDGE ring so they
    overlap the rf stream on the SP ring.
  Region B (PE-dense): AV supers (512-wide i blocks). The qf stripes are
    DMA'd and projected *inside* the AV matmul stream (stripe n+2 paced
    one channel-chunk per AV chunk of super n), and S/softmax for super
    n+1 is interleaved early in super n's window. P tiles are transposed
    by the DMA xbar (dma_start_transpose) into a per-super PTS tile
    [128j, 4il, 18jc, 128i] (contiguous destination); the AV matmuls read
    PTS[:, :, jc, :] as a strided 512-wide moving operand, so the PE
    spends zero cycles on P transposes.

Host side: shards 8 (batch, direction) problems over 8 cores, runs the
SPMD NEFF, and concatenates [orig, weighted] channel-wise.
"""

import sys

sys.path.insert(0, "/opt/trn_rl_repo")

import numpy as np

import concourse.bass as bass
import concourse.mybir as mybir
import concourse.tile as tile
from concourse import bacc
from concourse.bass_utils import run_bass_kernel_spmd
from concourse.masks import make_identity

B, C, H, W = 4, 2048, 48, 48
HW = H * W  # 2304
HC = 256

F32 = mybir.dt.float32
F32R = mybir.dt.float32r
BF16 = mybir.dt.bfloat16

NCC = C // 128  # 16 channel chunks
NHC = HC // 128  # 2 head-channel halves
NJT = HW // 128  # 18 j tiles
# Supers: i blocks for AV / qf projection stripes / S j-chunks / rf stripes.
SUP = [(0, 512), (512, 512), (1024, 512), (1536, 512), (2048, 256)]
NSUP = len(SUP)

_CACHED_NC = None


def build_nc(reps=1):
    nc = bacc.Bacc("TRN2", target_bir_lowering=False, debug=False, num_devices=8)

    qf = nc.dram_tensor("qf", [C, HW], F32, kind="ExternalInput").ap()
    rf = nc.dram_tensor("rf", [C, HW], F32, kind="ExternalInput").ap()
    Wq = nc.dram_tensor("Wq", [HC, C], F32, kind="ExternalInput").ap()
    bq = nc.dram_tensor("bq", [HC], F32, kind="ExternalInput").ap()
    Wk = nc.dram_tensor("Wk", [HC, C], F32, kind="ExternalInput").ap()
    bk = nc.dram_tensor("bk", [HC], F32, kind="ExternalInput").ap()
    out = nc.dram_tensor("out", [C, HW], BF16, kind="ExternalOutput").ap()

    with tile.TileContext(nc) as tc:
        for r in range(reps):
            build_tile_kernel(tc, out, qf, rf, Wq, bq, Wk, bk, rep=r)

    nc.compile()
    return nc


def tiles_of(n):
    off, ln = SUP[n]
    return list(range(off // 128, (off + ln) // 128))


def build_tile_kernel(tc, out, qf, rf, Wq, bq, Wk, bk, rep=0):
    nc = tc.nc

    with (
        tc.tile_pool(name="persist", bufs=1) as persist,
        tc.tile_pool(name="consts", bufs=1) as consts,
        tc.tile_pool(name="wq", bufs=1) as wq_pool,
    ):
        # Persistent tensors (live across phases).
        # VT[jp, cc, jc, ci] = rf[cc*128+ci, jc*128+jp] in bf16.
        VT = persist.tile([128, NCC, NJT, 128], BF16, tag="VT")
        Q_sb = persist.tile([128, NHC, HW], F32R, tag="Q")  # [hp, h, i]
        K_sb = persist.tile([128, NHC, HW], F32R, tag="K")  # [hp, h, j]
        WqT = wq_pool.tile([128, NCC, HC], F32R, tag="WqT")

        ident_f = consts.tile([128, 128], F32, tag="idf")
        ident_bf = consts.tile([128, 128], BF16, tag="idbf")
        make_identity(nc, ident_f[:])
        make_identity(nc, ident_bf[:])
        bq_t = consts.tile([128, NHC], F32, tag="bq")
        bk_t = consts.tile([128, NHC], F32, tag="bk")
        # Weight/bias loads ride the ACT HWDGE ring; the rf stream below is
        # on the SP ring, so they overlap.
        nc.scalar.dma_start(out=bq_t[:], in_=bq.rearrange("(h p) -> p h", p=128))
        nc.scalar.dma_start(out=bk_t[:], in_=bk.rearrange("(h p) -> p h", p=128))

        def proj_stripe(src, WT, dst, bias, s, with_vt, pools):
            """Project one 512-col stripe of src; optionally build VT."""
            xt_pool, xbf_pool, proj_psum, tr_psum = pools
            j0, jw = SUP[s]
            tagname = "k" if with_vt else "q"
            pp = [
                proj_psum.tile(
                    [128, jw], F32, tag=f"pp{h}", name=f"pp_{tagname}_{s}_{h}"
                )
                for h in range(NHC)
            ]
            for cc in range(NCC):
                xt = xt_pool.tile([128, jw], F32, tag="xt", name=f"xt{tagname}{s}{cc}")
                # Alternate HWDGE rings (SP/ACT) to halve dispatch latency.
                dma_eng = nc.sync if cc % 2 == 0 else nc.scalar
                dma_eng.dma_start(
                    out=xt[:], in_=src[cc * 128 : (cc + 1) * 128, j0 : j0 + jw]
                )
                xrt = xt_pool.tile(
                    [128, jw], F32R, tag="xr", name=f"xr{tagname}{s}{cc}"
                )
                nc.vector.tensor_copy(xrt[:], xt[:])
                xr = xrt[:]
                for h in range(NHC):
                    nc.tensor.matmul(
                        pp[h][:],
                        WT[:, cc, h * 128 : (h + 1) * 128],
                        xr,
                        start=(cc == 0),
                        stop=(cc == NCC - 1),
                    )
                if with_vt:
                    xbf = xbf_pool.tile([128, jw], BF16, tag="xbf", name=f"xbf{s}{cc}")
                    nc.vector.tensor_copy(xbf[:], xt[:])
                    njl = jw // 128
                    ptb = tr_psum.tile([128, 512], BF16, tag="vtp", name=f"vtp{s}{cc}")
                    for jl in range(njl):
                        # slices share one PSUM bank: only the first write may
                        # clear it (start=True)
                        nc.tensor.matmul(
                            ptb[:, jl * 128 : (jl + 1) * 128],
                            xbf[:, jl * 128 : (jl + 1) * 128],
                            ident_bf[:],
                            is_transpose=True,
                            start=(jl == 0),
                            stop=(jl == njl - 1),
                            skip_group_check=True,
                        )
                    jc0 = j0 // 128
                    dst_ap = VT[:, cc, jc0 : jc0 + njl, :]
                    src_ap = ptb[:, : njl * 128].rearrange("p (g b) -> p g b", g=njl)
                    nc.scalar.copy(dst_ap, src_ap)
            for h in range(NHC):
                nc.scalar.activation(
                    dst[:, h, j0 : j0 + jw],
                    pp[h][:],
                    mybir.ActivationFunctionType.Identity,
                    bias=bias[:, h : h + 1],
                    scale=1.0,
                )

        # ---- Phase W + Phase R (rf: K projection + VT build), WkT scoped.
        with tc.tile_pool(name="wk", bufs=1) as wk_pool:
            WkT = wk_pool.tile([128, NCC, HC], F32R, tag="WkT")
            with (
                tc.tile_pool(name="wraw", bufs=2) as wraw_pool,
                tc.tile_pool(name="wtpsum", bufs=4, space="PSUM") as wt_psum,
            ):
                for Wsrc, WT in ((Wk, WkT), (Wq, WqT)):
                    for h in range(NHC):
                        wr = wraw_pool.tile([128, C], F32, tag="wraw")
                        nc.scalar.dma_start(
                            out=wr[:], in_=Wsrc[h * 128 : (h + 1) * 128, :]
                        )
                        for cc in range(NCC):
                            pt = wt_psum.tile([128, 128], F32, tag="wtp")
                            nc.tensor.transpose(
                                pt[:], wr[:, cc * 128 : (cc + 1) * 128], ident_f[:]
                            )
                            nc.vector.tensor_copy(
                                WT[:, cc, h * 128 : (h + 1) * 128], pt[:]
                            )

            with (
                tc.tile_pool(name="xt", bufs=4) as xt_pool,
                tc.tile_pool(name="xbf", bufs=3) as xbf_pool,
                tc.tile_pool(name="ppsum", bufs=2, space="PSUM") as proj_psum,
                tc.tile_pool(name="trpsum", bufs=2, space="PSUM") as tr_psum,
            ):
                rpools = (xt_pool, xbf_pool, proj_psum, tr_psum)
                for s in range(NSUP):
                    proj_stripe(rf, WkT, K_sb, bk_t, s, True, rpools)

        # ---- Region B: qf stripes + S/softmax/PT + AV, all interleaved.
        with (
            tc.tile_pool(name="sbuf_s", bufs=2) as pool_s,
            tc.tile_pool(name="sbuf_p", bufs=2) as pool_p,
            tc.tile_pool(name="pts", bufs=2) as pts_pool,
            tc.tile_pool(name="sbuf_o", bufs=3) as pool_o,
            tc.tile_pool(name="small", bufs=4) as small,
            tc.tile_pool(name="xtq", bufs=3) as xtq_pool,
            tc.tile_pool(name="qpsum", bufs=1, space="PSUM") as q_psum,
            tc.tile_pool(name="spsum", bufs=3, space="PSUM") as s_psum,
            tc.tile_pool(name="opsum", bufs=3, space="PSUM") as o_psum,
        ):
            qpools = (xtq_pool, None, q_psum, None)
            PTs = {}

            def alloc_pts(n):
                nt = SUP[n][1] // 128
                PTs[n] = pts_pool.tile(
                    [128, nt, NJT, 128], BF16, tag="PTS", name=f"PTS_{n}"
                )

            def q_stripe_unit(s):
                """Return a per-cc closure emitting Q-projection chunk cc."""
                j0, jw = SUP[s]
                pp = [
                    q_psum.tile([128, jw], F32, tag=f"qp{h}", name=f"qpp_{s}_{h}")
                    for h in range(NHC)
                ]

                def unit(cc):
                    xt = xtq_pool.tile([128, jw], F32, tag="xt", name=f"xtq{s}{cc}")
                    nc.sync.dma_start(
                        out=xt[:], in_=qf[cc * 128 : (cc + 1) * 128, j0 : j0 + jw]
                    )
                    xrt = xtq_pool.tile([128, jw], F32R, tag="xr", name=f"xrq{s}{cc}")
                    nc.gpsimd.tensor_copy(xrt[:], xt[:])
                    xr = xrt[:]
                    for h in range(NHC):
                        nc.tensor.matmul(
                            pp[h][:],
                            WqT[:, cc, h * 128 : (h + 1) * 128],
                            xr,
                            start=(cc == 0),
                            stop=(cc == NCC - 1),
                        )
                    if cc == NCC - 1:
                        for h in range(NHC):
                            nc.scalar.activation(
                                Q_sb[:, h, j0 : j0 + jw],
                                pp[h][:],
                                mybir.ActivationFunctionType.Identity,
                                bias=bq_t[:, h : h + 1],
                                scale=1.0,
                            )

                return unit

            def s_work(it):
                """S matmuls + softmax + PT (DMA xbar transpose) for i-tile."""
                i0 = it * 128
                S_sb = pool_s.tile([128, HW], F32, tag="S", name=f"S_{it}")
                for jci, (j0, jn) in enumerate(SUP):
                    ps = s_psum.tile([128, 512], F32, tag="S", name=f"psS_{it}_{j0}")
                    for h in range(NHC):
                        nc.tensor.matmul(
                            ps[:, :jn],
                            Q_sb[:, h, i0 : i0 + 128],
                            K_sb[:, h, j0 : j0 + jn],
                            start=(h == 0),
                            stop=(h == NHC - 1),
                        )
                    nc.vector.tensor_copy(S_sb[:, j0 : j0 + jn], ps[:, :jn])
                negmax = small.tile([128, 1], F32, tag="negmax", name=f"nm_{it}")
                nc.vector.tensor_reduce(
                    negmax[:],
                    S_sb[:],
                    axis=mybir.AxisListType.X,
                    op=mybir.AluOpType.max,
                    negate=True,
                )
                P_bf = pool_p.tile([128, HW], BF16, tag="P", name=f"P_{it}")
                sumexp = small.tile([128, 1], F32, tag="sumexp", name=f"se_{it}")
                nc.scalar.activation(
                    P_bf[:],
                    S_sb[:],
                    mybir.ActivationFunctionType.Exp,
                    bias=negmax[:],
                    scale=1.0,
                    accum_out=sumexp[:],
                )
                rcp = small.tile([128, 1], F32, tag="rcp", name=f"rcp_{it}")
                nc.vector.reciprocal(rcp[:], sumexp[:])
                nc.vector.tensor_scalar_mul(P_bf[:], P_bf[:], rcp[:])
                n = it // 4
                il = it - 4 * n
                nc.sync.dma_start_transpose(out=PTs[n][:, il, :, :], in_=P_bf[:])

            def av_chunk(n, cc):
                sup_off, sup_len = SUP[n]
                PT = PTs[n]
                po = o_psum.tile([128, 512], F32, tag="O", name=f"psO_{n}_{cc}")
                for jc in range(NJT):
                    nc.tensor.matmul(
                        po[:, :sup_len],
                        VT[:, cc, jc, :],
                        PT[:, :, jc, :],
                        start=(jc == 0),
                        stop=(jc == NJT - 1),
                    )
                O_sb = pool_o.tile([128, 512], BF16, tag="O", name=f"O_{n}_{cc}")
                nc.scalar.copy(O_sb[:, :sup_len], po[:, :sup_len])
                nc.scalar.dma_start(
                    out=out[cc * 128 : (cc + 1) * 128, sup_off : sup_off + sup_len],
                    in_=O_sb[:, :sup_len],
                )

            # Prologue: Q stripe 0, then Q stripe 1 with S/softmax/PT of
            # super 0 interleaved (S needs only K + Q stripe 0).
            u0 = q_stripe_unit(0)
            for cc in range(NCC):
                u0(cc)
            alloc_pts(0)
            u1 = q_stripe_unit(1)
            for cc in range(NCC):
                u1(cc)
                if cc in (1, 4, 7, 10):
                    s_work(tiles_of(0)[(cc - 1) // 3])

            # Windows: AV(n) with Q stripe n+2 and S/PT of super n+1 paced in.
            # Q units run 2-per-chunk in the front half so their DMAs land
            # with slack; S tiles go at even chunks (Q stripe n+1 is ready).
            for n in range(NSUP):
                sched = {}
                if n + 2 < NSUP:
                    qu = q_stripe_unit(n + 2)
                    for cc in range(NCC):
                        sched.setdefault(cc // 2, []).append(("Q", qu, cc))
                if n + 1 < NSUP:
                    alloc_pts(n + 1)
                    for k, t in enumerate(tiles_of(n + 1)):
                        sched.setdefault(2 * k, []).append(("S", t, None))
                for cc in range(NCC):
                    av_chunk(n, cc)
                    for kind, a, b in sched.get(cc, []):
                        if kind == "Q":
                            a(b)
                        else:
                            s_work(a)


def get_nc():
    global _CACHED_NC
    if _CACHED_NC is None:
        _CACHED_NC = build_nc()
    return _CACHED_NC


def kernel(left_features, right_features, Wq, bq, Wk, bk):
    left = np.ascontiguousarray(np.asarray(left_features, dtype=np.float32)).reshape(
        B, C, HW
    )
    right = np.ascontiguousarray(np.asarray(right_features, dtype=np.float32)).reshape(
        B, C, HW
    )
    Wq = np.ascontiguousarray(np.asarray(Wq, dtype=np.float32))
    Wk = np.ascontiguousarray(np.asarray(Wk, dtype=np.float32))
    bq = np.ascontiguousarray(np.asarray(bq, dtype=np.float32))
    bk = np.ascontiguousarray(np.asarray(bk, dtype=np.float32))

    nc = get_nc()

    # cores 0..3: weighted_r for batch b (query=left, ref=right)
    # cores 4..7: weighted_l for batch b (query=right, ref=left)
    in_maps = []
    for b in range(B):
        in_maps.append(
            {"qf": left[b], "rf": right[b], "Wq": Wq, "bq": bq, "Wk": Wk, "bk": bk}
        )
    for b in range(B):
        in_maps.append(
            {"qf": right[b], "rf": left[b], "Wq": Wq, "bq": bq, "Wk": Wk, "bk": bk}
        )

    res = run_bass_kernel_spmd(nc, in_maps, core_ids=list(range(8)))

    weighted_r = np.stack(
        [np.asarray(res.results[b]["out"], dtype=np.float32) for b in range(B)]
    ).reshape(B, C, H, W)
    weighted_l = np.stack(
        [np.asarray(res.results[B + b]["out"], dtype=np.float32) for b in range(B)]
    ).reshape(B, C, H, W)
    left4 = left.reshape(B, C, H, W)
    right4 = right.reshape(B, C, H, W)
    left_attended = np.concatenate([left4, weighted_l], axis=1)
    right_attended = np.concatenate([right4, weighted_r], axis=1)
    return (left_attended, right_attended)


# revision 33
# speedup vs baseline: 2.8760x; 2.8760x over previous
"""CoAttention module kernel for Trainium2 (8 NeuronCores).

Problem: B=4 pairs of (left, right) feature maps [B, C=2048, H=W=48].
Two attention directions per pair -> 8 independent attention problems,
one per core (data parallel, no cross-core communication).

Per core (qf = query features [C, HW], rf = reference features [C, HW]):
    Q = Wq @ qf + bq          [HC=256, HW=2304]   (fp32r matmuls)
    K = Wk @ rf + bk          [HC=256, HW=2304]
    S = Q^T K                 [2304, 2304]        (fp32r)
    P = softmax(S, axis=-1)                       (exact row max, ACT exp)
    O = V P^T, V = rf         [C, HW]             (bf16 matmuls)

Schedule (emission order == per-engine execution order):
  Phase R (DMA-bound): rf streamed once in 512-col stripes; K projection
    (fp32r, PSUM-accumulated over the 16 channel chunks) plus the bf16
    V-transpose (PE transpose, hidden under the DMA stream). W^T tiles
    are built first from DMAs issued on the ACT HWDGE ring so they
    overlap the rf stream on the SP ring.
  Region B (PE-dense): AV supers (512-wide i blocks). The qf stripes are
    DMA'd and projected *inside* the AV matmul stream (stripe n+2 paced
    one channel-chunk per AV chunk of super n), and S/softmax for super
    n+1 is interleaved early in super n's window. P tiles are transposed
    by the DMA xbar (dma_start_transpose) into a per-super PTS tile
    [128j, 4il, 18jc, 128i] (contiguous destination); the AV matmuls read
    PTS[:, :, jc, :] as a strided 512-wide moving operand, so the PE
    spends zero cycles on P transposes.

Host side: shards 8 (batch, direction) problems over 8 cores, runs the
SPMD NEFF, and concatenates [orig, weighted] channel-wise.
"""

import sys

sys.path.insert(0, "/opt/trn_rl_repo")

import numpy as np

import concourse.bass as bass
import concourse.mybir as mybir
import concourse.tile as tile
from concourse import bacc
from concourse.bass_utils import run_bass_kernel_spmd
from concourse.masks import make_identity

B, C, H, W = 4, 2048, 48, 48
HW = H * W  # 2304
HC = 256

F32 = mybir.dt.float32
F32R = mybir.dt.float32r
BF16 = mybir.dt.bfloat16

NCC = C // 128  # 16 channel chunks
NHC = HC // 128  # 2 head-channel halves
NJT = HW // 128  # 18 j tiles
# Supers: i blocks for AV / qf projection stripes / S j-chunks / rf stripes.
SUP = [(0, 512), (512, 512), (1024, 512), (1536, 512), (2048, 256)]
NSUP = len(SUP)

_CACHED_NC = None


def build_nc(reps=1):
    nc = bacc.Bacc("TRN2", target_bir_lowering=False, debug=False, num_devices=8)

    qf = nc.dram_tensor("qf", [C, HW], F32, kind="ExternalInput").ap()
    rf = nc.dram_tensor("rf", [C, HW], F32, kind="ExternalInput").ap()
    Wq = nc.dram_tensor("Wq", [HC, C], F32, kind="ExternalInput").ap()
    bq = nc.dram_tensor("bq", [HC], F32, kind="ExternalInput").ap()
    Wk = nc.dram_tensor("Wk", [HC, C], F32, kind="ExternalInput").ap()
    bk = nc.dram_tensor("bk", [HC], F32, kind="ExternalInput").ap()
    out = nc.dram_tensor("out", [C, HW], BF16, kind="ExternalOutput").ap()

    with tile.TileContext(nc) as tc:
        for r in range(reps):
            build_tile_kernel(tc, out, qf, rf, Wq, bq, Wk, bk, rep=r)

    nc.compile()
    return nc


def tiles_of(n):
    off, ln = SUP[n]
    return list(range(off // 128, (off + ln) // 128))


def build_tile_kernel(tc, out, qf, rf, Wq, bq, Wk, bk, rep=0):
    nc = tc.nc

    with (
        tc.tile_pool(name="persist", bufs=1) as persist,
        tc.tile_pool(name="consts", bufs=1) as consts,
        tc.tile_pool(name="wq", bufs=1) as wq_pool,
    ):
        # Persistent tensors (live across phases).
        # VT[jp, cc, jc, ci] = rf[cc*128+ci, jc*128+jp] in bf16.
        VT = persist.tile([128, NCC, NJT, 128], BF16, tag="VT")
        Q_sb = persist.tile([128, NHC, HW], F32R, tag="Q")  # [hp, h, i]
        K_sb = persist.tile([128, NHC, HW], F32R, tag="K")  # [hp, h, j]
        WqT = wq_pool.tile([128, NCC, HC], F32R, tag="WqT")

        ident_f = consts.tile([128, 128], F32, tag="idf")
        ident_bf = consts.tile([128, 128], BF16, tag="idbf")
        make_identity(nc, ident_f[:])
        make_identity(nc, ident_bf[:])
        bq_t = consts.tile([128, NHC], F32, tag="bq")
        bk_t = consts.tile([128, NHC], F32, tag="bk")
        # Weight/bias loads ride the ACT HWDGE ring; the rf stream below is
        # on the SP ring, so they overlap.
        nc.scalar.dma_start(out=bq_t[:], in_=bq.rearrange("(h p) -> p h", p=128))
        nc.scalar.dma_start(out=bk_t[:], in_=bk.rearrange("(h p) -> p h", p=128))

        def proj_stripe(src, WT, dst, bias, s, with_vt, pools):
            """Project one 512-col stripe of src; optionally build VT."""
            xt_pool, xbf_pool, proj_psum, tr_psum = pools
            j0, jw = SUP[s]
            tagname = "k" if with_vt else "q"
            pp = [
                proj_psum.tile(
                    [128, jw], F32, tag=f"pp{h}", name=f"pp_{tagname}_{s}_{h}"
                )
                for h in range(NHC)
            ]
            for cc in range(NCC):
                xt = xt_pool.tile([128, jw], F32, tag="xt", name=f"xt{tagname}{s}{cc}")
                nc.sync.dma_start(
                    out=xt[:], in_=src[cc * 128 : (cc + 1) * 128, j0 : j0 + jw]
                )
                xrt = xt_pool.tile(
                    [128, jw], F32R, tag="xr", name=f"xr{tagname}{s}{cc}"
                )
                nc.vector.tensor_copy(xrt[:], xt[:])
                xr = xrt[:]
                for h in range(NHC):
                    nc.tensor.matmul(
                        pp[h][:],
                        WT[:, cc, h * 128 : (h + 1) * 128],
                        xr,
                        start=(cc == 0),
                        stop=(cc == NCC - 1),
                    )
                if with_vt:
                    xbf = xbf_pool.tile([128, jw], BF16, tag="xbf", name=f"xbf{s}{cc}")
                    nc.vector.tensor_copy(xbf[:], xt[:])
                    njl = jw // 128
                    ptb = tr_psum.tile([128, 512], BF16, tag="vtp", name=f"vtp{s}{cc}")
                    for jl in range(njl):
                        # slices share one PSUM bank: only the first write may
                        # clear it (start=True)
                        nc.tensor.matmul(
                            ptb[:, jl * 128 : (jl + 1) * 128],
                            xbf[:, jl * 128 : (jl + 1) * 128],
                            ident_bf[:],
                            is_transpose=True,
                            start=(jl == 0),
                            stop=(jl == njl - 1),
                            skip_group_check=True,
                        )
                    jc0 = j0 // 128
                    dst_ap = VT[:, cc, jc0 : jc0 + njl, :]
                    src_ap = ptb[:, : njl * 128].rearrange("p (g b) -> p g b", g=njl)
                    if cc % 2 == 0:
                        nc.scalar.copy(dst_ap, src_ap)
                    else:
                        nc.vector.tensor_copy(dst_ap, src_ap)
            for h in range(NHC):
                nc.scalar.activation(
                    dst[:, h, j0 : j0 + jw],
                    pp[h][:],
                    mybir.ActivationFunctionType.Identity,
                    bias=bias[:, h : h + 1],
                    scale=1.0,
                )

        # ---- Phase W + Phase R (rf: K projection + VT build), WkT scoped.
        # Wk is loaded/transposed first (K projection gates everything); the
        # Wq load+transpose is deferred until after rf stripe 0 so the rf
        # stream starts as early as possible.
        with tc.tile_pool(name="wk", bufs=1) as wk_pool:
            WkT = wk_pool.tile([128, NCC, HC], F32R, tag="WkT")
            with (
                tc.tile_pool(name="wraw", bufs=2) as wraw_pool,
                tc.tile_pool(name="wtpsum", bufs=4, space="PSUM") as wt_psum,
                tc.tile_pool(name="xt", bufs=4) as xt_pool,
                tc.tile_pool(name="xbf", bufs=3) as xbf_pool,
                tc.tile_pool(name="ppsum", bufs=2, space="PSUM") as proj_psum,
                tc.tile_pool(name="trpsum", bufs=2, space="PSUM") as tr_psum,
            ):

                def build_wt(Wsrc, WT):
                    for h in range(NHC):
                        wr = wraw_pool.tile([128, C], F32, tag="wraw")
                        nc.scalar.dma_start(
                            out=wr[:], in_=Wsrc[h * 128 : (h + 1) * 128, :]
                        )
                        for cc in range(NCC):
                            pt = wt_psum.tile([128, 128], F32, tag="wtp")
                            nc.tensor.transpose(
                                pt[:], wr[:, cc * 128 : (cc + 1) * 128], ident_f[:]
                            )
                            nc.vector.tensor_copy(
                                WT[:, cc, h * 128 : (h + 1) * 128], pt[:]
                            )

                rpools = (xt_pool, xbf_pool, proj_psum, tr_psum)
                build_wt(Wk, WkT)
                for s in range(NSUP):
                    proj_stripe(rf, WkT, K_sb, bk_t, s, True, rpools)
                    if s == 0:
                        build_wt(Wq, WqT)

        # ---- Region B: qf stripes + S/softmax/PT + AV, all interleaved.
        with (
            tc.tile_pool(name="sbuf_s", bufs=2) as pool_s,
            tc.tile_pool(name="sbuf_p", bufs=2) as pool_p,
            tc.tile_pool(name="pts", bufs=2) as pts_pool,
            tc.tile_pool(name="sbuf_o", bufs=3) as pool_o,
            tc.tile_pool(name="small", bufs=4) as small,
            tc.tile_pool(name="xtq", bufs=3) as xtq_pool,
            tc.tile_pool(name="qpsum", bufs=1, space="PSUM") as q_psum,
            tc.tile_pool(name="spsum", bufs=2, space="PSUM") as s_psum,
            tc.tile_pool(name="opsum", bufs=2, space="PSUM") as o_psum,
            tc.tile_pool(name="ptpsum", bufs=2, space="PSUM") as p_psum,
        ):
            qpools = (xtq_pool, None, q_psum, None)
            PTs = {}

            def alloc_pts(n):
                PTs[n] = pts_pool.tile(
                    [128, NJT, 512], BF16, tag="PTS", name=f"PTS_{n}"
                )

            def q_stripe_unit(s):
                """Return a per-cc closure emitting Q-projection chunk cc."""
                j0, jw = SUP[s]
                pp = [
                    q_psum.tile([128, jw], F32, tag=f"qp{h}", name=f"qpp_{s}_{h}")
                    for h in range(NHC)
                ]

                def unit(cc):
                    xt = xtq_pool.tile([128, jw], F32, tag="xt", name=f"xtq{s}{cc}")
                    nc.sync.dma_start(
                        out=xt[:], in_=qf[cc * 128 : (cc + 1) * 128, j0 : j0 + jw]
                    )
                    xrt = xtq_pool.tile([128, jw], F32R, tag="xr", name=f"xrq{s}{cc}")
                    nc.vector.tensor_copy(xrt[:], xt[:])
                    xr = xrt[:]
                    for h in range(NHC):
                        nc.tensor.matmul(
                            pp[h][:],
                            WqT[:, cc, h * 128 : (h + 1) * 128],
                            xr,
                            start=(cc == 0),
                            stop=(cc == NCC - 1),
                        )
                    if cc == NCC - 1:
                        for h in range(NHC):
                            nc.scalar.activation(
                                Q_sb[:, h, j0 : j0 + jw],
                                pp[h][:],
                                mybir.ActivationFunctionType.Identity,
                                bias=bq_t[:, h : h + 1],
                                scale=1.0,
                            )

                return unit

            def s_work(it):
                """S matmuls + softmax + PT (DMA xbar transpose) for i-tile."""
                i0 = it * 128
                S_sb = pool_s.tile([128, HW], F32, tag="S", name=f"S_{it}")
                for jci, (j0, jn) in enumerate(SUP):
                    ps = s_psum.tile([128, 512], F32, tag="S", name=f"psS_{it}_{j0}")
                    for h in range(NHC):
                        nc.tensor.matmul(
                            ps[:, :jn],
                            Q_sb[:, h, i0 : i0 + 128],
                            K_sb[:, h, j0 : j0 + jn],
                            start=(h == 0),
                            stop=(h == NHC - 1),
                        )
                    nc.vector.tensor_copy(S_sb[:, j0 : j0 + jn], ps[:, :jn])
                negmax = small.tile([128, 1], F32, tag="negmax", name=f"nm_{it}")
                nc.vector.tensor_reduce(
                    negmax[:],
                    S_sb[:],
                    axis=mybir.AxisListType.X,
                    op=mybir.AluOpType.max,
                    negate=True,
                )
                P_bf = pool_p.tile([128, HW], BF16, tag="P", name=f"P_{it}")
                sumexp = small.tile([128, 1], F32, tag="sumexp", name=f"se_{it}")
                nc.scalar.activation(
                    P_bf[:],
                    S_sb[:],
                    mybir.ActivationFunctionType.Exp,
                    bias=negmax[:],
                    scale=1.0,
                    accum_out=sumexp[:],
                )
                rcp = small.tile([128, 1], F32, tag="rcp", name=f"rcp_{it}")
                nc.vector.reciprocal(rcp[:], sumexp[:])
                nc.vector.tensor_scalar_mul(P_bf[:], P_bf[:], rcp[:])
                n = it // 4
                il = (it - 4 * n) * 128
                # PE transpose of P into PTs[n][:, jc, il:il+128], in groups
                # of 4 sharing one PSUM bank.
                for g0 in range(0, NJT, 4):
                    gn = min(4, NJT - g0)
                    ptb = p_psum.tile(
                        [128, 512], BF16, tag="ptp", name=f"ptp_{it}_{g0}"
                    )
                    for jl in range(g0, g0 + gn):
                        nc.tensor.matmul(
                            ptb[:, (jl - g0) * 128 : (jl - g0 + 1) * 128],
                            P_bf[:, jl * 128 : (jl + 1) * 128],
                            ident_bf[:],
                            is_transpose=True,
                            start=(jl == g0),
                            stop=(jl == g0 + gn - 1),
                            skip_group_check=True,
                        )
                    dst = PTs[n][:, g0 : g0 + gn, il : il + 128]
                    src = ptb[:, : gn * 128].rearrange("p (g b) -> p g b", g=gn)
                    if g0 % 8 == 0:
                        nc.scalar.copy(dst, src)
                    else:
                        nc.vector.tensor_copy(dst, src)

            def av_chunk(n, cc):
                sup_off, sup_len = SUP[n]
                PT = PTs[n]
                po = o_psum.tile([128, 512], F32, tag="O", name=f"psO_{n}_{cc}")
                for jc in range(NJT):
                    nc.tensor.matmul(
                        po[:, :sup_len],
                        VT[:, cc, jc, :],
                        PT[:, jc, :sup_len],
                        start=(jc == 0),
                        stop=(jc == NJT - 1),
                    )
                O_sb = pool_o.tile([128, 512], BF16, tag="O", name=f"O_{n}_{cc}")
                nc.scalar.copy(O_sb[:, :sup_len], po[:, :sup_len])
                nc.sync.dma_start(
                    out=out[cc * 128 : (cc + 1) * 128, sup_off : sup_off + sup_len],
                    in_=O_sb[:, :sup_len],
                )

            # Prologue: Q stripe 0, then Q stripe 1 with S/softmax/PT of
            # super 0 interleaved (S needs only K + Q stripe 0).
            u0 = q_stripe_unit(0)
            for cc in range(NCC):
                u0(cc)
            alloc_pts(0)
            u1 = q_stripe_unit(1)
            for cc in range(NCC):
                u1(cc)
                if cc in (1, 4, 7, 10):
                    s_work(tiles_of(0)[(cc - 1) // 3])

            # Windows: AV(n) with Q stripe n+2 and S/PT of super n+1 paced in.
            # Q units run 2-per-chunk in the front half so their DMAs land
            # with slack; S tiles go at even chunks (Q stripe n+1 is ready).
            for n in range(NSUP):
                sched = {}
                if n + 2 < NSUP:
                    qu = q_stripe_unit(n + 2)
                    for cc in range(NCC):
                        sched.setdefault(cc // 2, []).append(("Q", qu, cc))
                if n + 1 < NSUP:
                    alloc_pts(n + 1)
                    for k, t in enumerate(tiles_of(n + 1)):
                        sched.setdefault(2 * k, []).append(("S", t, None))
                for cc in range(NCC):
                    av_chunk(n, cc)
                    for kind, a, b in sched.get(cc, []):
                        if kind == "Q":
                            a(b)
                        else:
                            s_work(a)


def get_nc():
    global _CACHED_NC
    if _CACHED_NC is None:
        _CACHED_NC = build_nc()
    return _CACHED_NC


def kernel(left_features, right_features, Wq, bq, Wk, bk):
    left = np.ascontiguousarray(np.asarray(left_features, dtype=np.float32)).reshape(
        B, C, HW
    )
    right = np.ascontiguousarray(np.asarray(right_features, dtype=np.float32)).reshape(
        B, C, HW
    )
    Wq = np.ascontiguousarray(np.asarray(Wq, dtype=np.float32))
    Wk = np.ascontiguousarray(np.asarray(Wk, dtype=np.float32))
    bq = np.ascontiguousarray(np.asarray(bq, dtype=np.float32))
    bk = np.ascontiguousarray(np.asarray(bk, dtype=np.float32))

    nc = get_nc()

    # cores 0..3: weighted_r for batch b (query=left, ref=right)
    # cores 4..7: weighted_l for batch b (query=right, ref=left)
    in_maps = []
    for b in range(B):
        in_maps.append(
            {"qf": left[b], "rf": right[b], "Wq": Wq, "bq": bq, "Wk": Wk, "bk": bk}
        )
    for b in range(B):
        in_maps.append(
            {"qf": right[b], "rf": left[b], "Wq": Wq, "bq": bq, "Wk": Wk, "bk": bk}
        )

    res = run_bass_kernel_spmd(nc, in_maps, core_ids=list(range(8)))

    weighted_r = np.stack(
        [np.asarray(res.results[b]["out"], dtype=np.float32) for b in range(B)]
    ).reshape(B, C, H, W)
    weighted_l = np.stack(
        [np.asarray(res.results[B + b]["out"], dtype=np.float32) for b in range(B)]
    ).reshape(B, C, H, W)
    left4 = left.reshape(B, C, H, W)
    right4 = right.reshape(B, C, H, W)
    left_attended = np.concatenate([left4, weighted_l], axis=1)
    right_attended = np.concatenate([right4, weighted_r], axis=1)
    return (left_attended, right_attended)
